# revision 10
# baseline (speedup 1.0000x reference)
"""GatedDeltaNet forward on 8 trn2 NeuronCores (Bass/Tile, SPMD).

Sharding: v-heads across cores (2 v-heads + their shared k-head per core).
Per core: qkvz/ba projection slice -> causal depthwise conv -> l2 norms ->
chunkwise gated delta rule (WY representation, Horner order-3 triangular
solve) -> gated RMSNorm -> partial o_proj. Host sums the 8 partials.
"""
import numpy as np
import ml_dtypes
from contextlib import ExitStack

import concourse.bacc as bacc
import concourse.bass as bass
import concourse.mybir as mybir
import concourse.tile as tile
from concourse.bass_utils import run_bass_kernel_spmd

F32 = mybir.dt.float32
BF16 = mybir.dt.bfloat16
AF = mybir.ActivationFunctionType

B, T, H = 1, 1024, 2048
HK, HV, DK, DV, K = 8, 16, 128, 128, 4
KD, VD = HK * DK, HV * DV
EPS = 1e-6
NCORE = 8
C = 128                 # chunk length
NCH = T // C            # 8 chunks
NH = 16                 # 16 h-slices of the H=2048 contraction
WCOLS = 896             # 7 col-chunks of stage-A output (772 used, padded)
# col-chunk roles: 0=q 1=k 2=v0 3=v1 4=z0 5=z1 6=ba(+pad)


def _build_nc():
    nc = bacc.Bacc("TRN2", target_bir_lowering=False, debug=False,
                   num_devices=NCORE)
    d = {}
    d["xT"] = nc.dram_tensor("xT", [H, T], BF16, kind="ExternalInput")
    d["wq"] = nc.dram_tensor("wq", [H, WCOLS], BF16, kind="ExternalInput")
    d["convw"] = nc.dram_tensor("convw", [512, K], F32, kind="ExternalInput")
    d["convb"] = nc.dram_tensor("convb", [512, 1], F32, kind="ExternalInput")
    d["negea"] = nc.dram_tensor("negea", [2, 1], F32, kind="ExternalInput")
    d["dtb"] = nc.dram_tensor("dtb", [2, 1], F32, kind="ExternalInput")
    d["eye"] = nc.dram_tensor("eye", [128, 128], F32, kind="ExternalInput")
    d["cums"] = nc.dram_tensor("cums", [128, 128], F32, kind="ExternalInput")
    d["mincl"] = nc.dram_tensor("mincl", [128, 256], F32, kind="ExternalInput")
    d["msneg"] = nc.dram_tensor("msneg", [128, 256], F32, kind="ExternalInput")
    d["onesc"] = nc.dram_tensor("onesc", [128, 1], F32, kind="ExternalInput")
    d["onesr"] = nc.dram_tensor("onesr", [1, 128], F32, kind="ExternalInput")
    d["qscr"] = nc.dram_tensor("qscr", [1, 128], F32, kind="ExternalInput")
    d["normw"] = nc.dram_tensor("normw", [128, 1], F32, kind="ExternalInput")
    d["epsc"] = nc.dram_tensor("epsc", [128, 1], F32, kind="ExternalInput")
    d["wo"] = nc.dram_tensor("wo", [256, H], BF16, kind="ExternalInput")
    out = nc.dram_tensor("out_part", [T, H], F32, kind="ExternalOutput")

    with tile.TileContext(nc) as tc, ExitStack() as ctx:
        _emit(ctx, tc, d, out)
    nc.compile()
    return nc


def _emit(ctx, tc, d, out):
    nc = tc.nc
    mm = nc.tensor.matmul
    act = nc.scalar.activation

    cst = ctx.enter_context(tc.tile_pool(name="cst", bufs=1))
    xp = ctx.enter_context(tc.tile_pool(name="xp", bufs=1))
    wp = ctx.enter_context(tc.tile_pool(name="wp", bufs=3))
    big = ctx.enter_context(tc.tile_pool(name="big", bufs=1))
    mx = ctx.enter_context(tc.tile_pool(name="mx", bufs=2))
    rs = ctx.enter_context(tc.tile_pool(name="rs", bufs=2))
    ob = ctx.enter_context(tc.tile_pool(name="ob", bufs=2))
    one = ctx.enter_context(tc.tile_pool(name="one", bufs=1))
    # PSUM: 8 banks total.  "A": 3x [128,512] banks.  "r": 3x [128,256]
    # (padded to bank).  "t": 2x small (max [1,512] = 1 bank).
    psb = ctx.enter_context(tc.tile_pool(name="psb", bufs=3, space="PSUM"))
    psr = ctx.enter_context(tc.tile_pool(name="psr", bufs=3, space="PSUM"))
    pst = ctx.enter_context(tc.tile_pool(name="pst", bufs=2, space="PSUM"))

    _cnt = [0]

    def rtile(shape=(128, 256)):
        _cnt[0] += 1
        return psr.tile(list(shape), F32, tag="r", name=f"r{_cnt[0]}")

    def ttile(shape):
        _cnt[0] += 1
        return pst.tile(list(shape), F32, tag="t", name=f"t{_cnt[0]}")

    # ---- constants / small inputs ----
    def c_load(name, shape, dt=F32):
        t = cst.tile(shape, dt, tag=name, name=name)
        nc.sync.dma_start(t[:], d[name][:])
        return t
    eye = c_load("eye", [128, 128])
    cums = c_load("cums", [128, 128])
    mincl = c_load("mincl", [128, 256])
    msneg = c_load("msneg", [128, 256])
    onesc = c_load("onesc", [128, 1])
    onesr = c_load("onesr", [1, 128])
    qscr = c_load("qscr", [1, 128])
    normw = c_load("normw", [128, 1])
    epsc = c_load("epsc", [128, 1])
    negea = c_load("negea", [2, 1])
    dtb = c_load("dtb", [2, 1])
    convw = cst.tile([128, 4, K], F32, tag="convw")
    nc.sync.dma_start(convw[:],
                      d["convw"][:].rearrange("(c p) k -> p c k", p=128))
    convb = cst.tile([128, 4, 1], F32, tag="convb")
    nc.sync.dma_start(convb[:],
                      d["convb"][:].rearrange("(c p) k -> p c k", p=128))
    wo = cst.tile([128, 2, H], BF16, tag="wo")
    nc.sync.dma_start(wo[:], d["wo"][:].rearrange("(h p) n -> p h n", p=128))

    # ---- resident xT ----
    xt = []
    for s in range(NH):
        t = xp.tile([128, T], BF16, tag=f"xt{s}")
        nc.sync.dma_start(t[:], d["xT"][s * 128:(s + 1) * 128, :])
        xt.append(t)

    # ---- persistent stage-A products ----
    qn = big.tile([128, T], F32, tag="qn")      # normalized q^T
    kn = big.tile([128, T], F32, tag="kn")      # normalized k^T
    v0 = big.tile([128, T], F32, tag="v0")
    v1 = big.tile([128, T], F32, tag="v1")
    zs0 = big.tile([128, T], F32, tag="zs0")    # silu(z) head 0
    zs1 = big.tile([128, T], F32, tag="zs1")
    betaT = big.tile([2, T], F32, tag="betaT")
    beta1 = big.tile([1, T], F32, tag="beta1")  # head-1 row at partition 0
    gT = big.tile([2, T], F32, tag="gT")
    s0 = big.tile([128, 256], F32, tag="s0")    # recurrent state, 2 heads
    nc.vector.memset(s0[:], 0.0)
    o_all = big.tile([128, NCH * 256], F32, tag="o_all")

    # ---- stage A: qkvz/ba projection, channel-major out [col, T] ----
    def proj(cc):
        wt = wp.tile([128, NH, 128], BF16, tag="wt")
        nc.sync.dma_start(
            wt[:], d["wq"][:, cc * 128:(cc + 1) * 128]
            .rearrange("(s p) m -> p s m", p=128))
        ps0 = psb.tile([128, 512], F32, tag="A")
        ps1 = psb.tile([128, 512], F32, tag="A")
        for s in range(NH):
            mm(ps0[:], wt[:, s, :], xt[s][:, 0:512],
               start=(s == 0), stop=(s == NH - 1))
        for s in range(NH):
            mm(ps1[:], wt[:, s, :], xt[s][:, 512:1024],
               start=(s == 0), stop=(s == NH - 1))
        return ps0, ps1

    # ba first (gates feed the recurrence chain)
    # wq col layout in chunk 6: p0,p1 = b0,b1 ; p32 = b1 dup ; p64,p65 = a0,a1
    ps0, ps1 = proj(6)
    spT = one.tile([2, T], F32, tag="spT")
    abx = one.tile([2, T], F32, tag="abx")
    for half, ps in ((0, ps0), (1, ps1)):
        fs = slice(half * 512, (half + 1) * 512)
        act(betaT[:, fs], ps[0:2, :], AF.Sigmoid)
        act(beta1[:, fs], ps[32:33, :], AF.Sigmoid)
        act(abx[:, fs], ps[64:66, :], AF.Abs, bias=dtb[:, 0:1])
        act(spT[:, fs], ps[64:66, :], AF.Relu, bias=dtb[:, 0:1])
    act(abx[:], abx[:], AF.Exp, scale=-1.0)
    act(abx[:], abx[:], AF.Ln, bias=1.0)
    nc.vector.tensor_add(spT[:], spT[:], abx[:])
    nc.vector.tensor_scalar_mul(gT[:], spT[:], negea[:, 0:1])

    # q|k|v with conv+silu
    def conv_silu(cc, dst):
        ps0, ps1 = proj(cc)
        mixt = mx.tile([128, T + 3], F32, tag="mix")
        nc.vector.memset(mixt[:, 0:3], 0.0)
        nc.vector.tensor_copy(mixt[:, 3:515], ps0[:])
        nc.vector.tensor_copy(mixt[:, 515:1027], ps1[:])
        w = convw[:, cc, :]
        p0 = one.tile([128, T], F32, tag="p0")
        p1 = one.tile([128, T], F32, tag="p1")
        nc.vector.tensor_scalar_mul(p0[:], mixt[:, 0:T], w[:, 0:1])
        nc.vector.tensor_scalar_mul(p1[:], mixt[:, 1:T + 1], w[:, 1:2])
        nc.vector.tensor_add(p0[:], p0[:], p1[:])
        nc.vector.tensor_scalar_mul(p1[:], mixt[:, 2:T + 2], w[:, 2:3])
        nc.vector.tensor_add(p0[:], p0[:], p1[:])
        nc.vector.tensor_scalar_mul(p1[:], mixt[:, 3:T + 3], w[:, 3:4])
        nc.vector.tensor_add(p0[:], p0[:], p1[:])
        # silu(y + b) = (y+b) * sigmoid(y+b)
        act(p1[:], p0[:], AF.Sigmoid, bias=convb[:, cc, :])
        nc.vector.tensor_scalar(p0[:], p0[:], convb[:, cc, :], None,
                                op0=mybir.AluOpType.add)
        nc.vector.tensor_mul(dst[:], p0[:], p1[:])

    conv_silu(0, qn)
    conv_silu(1, kn)
    conv_silu(2, v0)
    conv_silu(3, v1)

    # l2 norms over dk (partition axis); q also gets dk^-0.5
    def l2norm(src, dst, repl):
        sq = one.tile([128, T], F32, tag="sq")
        nc.vector.tensor_mul(sq[:], src[:], src[:])
        rsq = one.tile([1, T], F32, tag="rsq")
        for h2 in range(2):
            fs = slice(h2 * 512, (h2 + 1) * 512)
            ssp = ttile([1, 512])
            mm(ssp[:], onesc[:], sq[:, fs])
            act(rsq[:, fs], ssp[:], AF.Sqrt, bias=epsc[0:1, :])
        nc.vector.reciprocal(rsq[:], rsq[:])
        for h2 in range(2):
            fs = slice(h2 * 512, (h2 + 1) * 512)
            rep = psb.tile([128, 512], F32, tag="A")
            mm(rep[:], repl[:], rsq[:, fs])
            nc.vector.tensor_mul(dst[:, fs], src[:, fs], rep[:])
    l2norm(qn, qn, qscr)
    l2norm(kn, kn, onesr)

    # z silu
    zsg = one.tile([128, 512], F32, tag="zsg")
    for cc, dst in ((4, zs0), (5, zs1)):
        ps0, ps1 = proj(cc)
        for half, ps in ((0, ps0), (1, ps1)):
            fs = slice(half * 512, (half + 1) * 512)
            act(zsg[:], ps[:], AF.Sigmoid)
            nc.vector.tensor_mul(dst[:, fs], ps[:], zsg[:])

    # ---- recurrence over chunks ----
    for ci in range(NCH):
        sl = slice(ci * C, (ci + 1) * C)
        # gate prep: G = within-chunk inclusive cumsum of g (time-major)
        gtm_ps = ttile([128, 2])
        nc.tensor.transpose(gtm_ps[:], gT[:, sl], eye[0:2, 0:2])
        gtm = rs.tile([128, 2], F32, tag="gtm")
        nc.vector.tensor_copy(gtm[:], gtm_ps[:])
        G_ps = ttile([128, 2])
        mm(G_ps[:], cums[:], gtm[:])
        G = rs.tile([128, 2], F32, tag="G")
        nc.vector.tensor_copy(G[:], G_ps[:])
        cc_ = rs.tile([128, 2], F32, tag="cc_")
        act(cc_[:], G[:], AF.Exp)
        btm_ps = ttile([128, 2])
        nc.tensor.transpose(btm_ps[:], betaT[:, sl], eye[0:2, 0:2])
        btm = rs.tile([128, 2], F32, tag="btm")
        nc.vector.tensor_copy(btm[:], btm_ps[:])
        cb = rs.tile([128, 2], F32, tag="cb")
        nc.vector.tensor_mul(cb[:], cc_[:], btm[:])
        # row-form G per head (partition 0)
        grows = []
        for h in range(2):
            gr_ps = ttile([1, 128])
            nc.tensor.transpose(gr_ps[:], G[:, h:h + 1], eye[:])
            gw = rs.tile([1, 128], F32, tag=f"grow{h}")
            nc.vector.tensor_copy(gw[:], gr_ps[:])
            grows.append(gw)

        # decay matrices in transposed layout [j, i], per head
        E = rs.tile([128, 256], F32, tag="E")
        for h in range(2):
            grp = rtile()
            mm(grp[:, 0:128], onesr[:], grows[h][:])
            nc.vector.tensor_scalar(E[:, h * 128:(h + 1) * 128],
                                    grp[:, 0:128], G[:, h:h + 1], None,
                                    op0=mybir.AluOpType.subtract)
        nc.vector.tensor_scalar_min(E[:], E[:], 0.0)
        dx = rs.tile([128, 256], F32, tag="dx")
        act(dx[:], E[:], AF.Exp)
        dm = rs.tile([128, 256], F32, tag="dm")
        nc.vector.tensor_mul(dm[:], dx[:], mincl[:])

        kkq_ps = rtile()            # [KK | KQ]
        mm(kkq_ps[:, 0:128], kn[:, sl], kn[:, sl])
        mm(kkq_ps[:, 128:256], kn[:, sl], qn[:, sl])
        attnT = rs.tile([128, 256], F32, tag="attnT")
        t1 = rs.tile([128, 256], F32, tag="t1")
        for h in range(2):
            hs = slice(h * 128, (h + 1) * 128)
            nc.vector.tensor_mul(attnT[:, hs], kkq_ps[:, 128:256], dm[:, hs])
            nc.vector.tensor_mul(t1[:, hs], kkq_ps[:, 0:128], dm[:, hs])
        brep = rtile()
        mm(brep[:, 0:128], onesr[:], betaT[0:1, sl])
        mm(brep[:, 128:256], onesr[:], beta1[0:1, sl])
        pt = rs.tile([128, 256], F32, tag="pt")
        nc.vector.tensor_mul(pt[:], brep[:], msneg[:])
        nc.vector.tensor_mul(pt[:], t1[:], pt[:])

        # R = beta*V - (c*beta)*(K @ S0)   [C, dv] per head
        vtm_ps = rtile()
        nc.tensor.transpose(vtm_ps[:, 0:128], v0[:, sl], eye[:])
        nc.tensor.transpose(vtm_ps[:, 128:256], v1[:, sl], eye[:])
        ks_ps = rtile()
        for h in range(2):
            hs = slice(h * 128, (h + 1) * 128)
            mm(ks_ps[:, hs], kn[:, sl], s0[:, hs])
        R = rs.tile([128, 256], F32, tag="R")
        tv = rs.tile([128, 256], F32, tag="tv")
        for h in range(2):
            hs = slice(h * 128, (h + 1) * 128)
            nc.vector.tensor_scalar_mul(tv[:, hs], vtm_ps[:, hs],
                                        btm[:, h:h + 1])
            nc.vector.tensor_scalar_mul(R[:, hs], ks_ps[:, hs],
                                        cb[:, h:h + 1])
        nc.vector.tensor_sub(R[:], tv[:], R[:])

        # Horner: U = R + P(R + P(R + P R))
        X = R
        for it in range(3):
            hp = rtile()
            for h in range(2):
                hs = slice(h * 128, (h + 1) * 128)
                mm(hp[:, hs], pt[:, hs], X[:, hs])
            Xn = rs.tile([128, 256], F32, tag=f"X{it}")
            nc.vector.tensor_add(Xn[:], hp[:], R[:])
            X = Xn
        U = X

        # O = c*(Q @ S0) + Attn @ U
        qs_ps = rtile()
        au_ps = rtile()
        for h in range(2):
            hs = slice(h * 128, (h + 1) * 128)
            mm(qs_ps[:, hs], qn[:, sl], s0[:, hs])
            mm(au_ps[:, hs], attnT[:, hs], U[:, hs])
        O = o_all[:, ci * 256:(ci + 1) * 256]
        for h in range(2):
            hs = slice(h * 128, (h + 1) * 128)
            nc.vector.tensor_scalar_mul(O[:, hs], qs_ps[:, hs],
                                        cc_[:, h:h + 1])
        nc.vector.tensor_add(O[:], O[:], au_ps[:])

        # state: S = cC*S0 + (cr*K)^T @ U ; cC = c_p * dm[p, last] any p
        ktm_ps = rtile()
        nc.tensor.transpose(ktm_ps[:, 0:128], kn[:, sl], eye[:])
        ksc = rs.tile([128, 256], F32, tag="ksc")
        ccol = rs.tile([128, 2], F32, tag="ccol")
        for h in range(2):
            hs = slice(h * 128, (h + 1) * 128)
            nc.vector.tensor_scalar_mul(ksc[:, hs], ktm_ps[:, 0:128],
                                        dm[:, h * 128 + 127:h * 128 + 128])
            nc.vector.tensor_mul(ccol[:, h:h + 1], cc_[:, h:h + 1],
                                 dm[:, h * 128 + 127:h * 128 + 128])
        sn_ps = rtile()
        ci_sb = rs.tile([128, 256], F32, tag="ci_sb")
        for h in range(2):
            hs = slice(h * 128, (h + 1) * 128)
            nc.vector.tensor_scalar_mul(ci_sb[:, hs], eye[:],
                                        ccol[:, h:h + 1])
            mm(sn_ps[:, hs], ksc[:, hs], U[:, hs], start=True, stop=False)
            mm(sn_ps[:, hs], ci_sb[:, hs], s0[:, hs], start=False, stop=True)
        nc.vector.tensor_copy(s0[:], sn_ps[:])

    # ---- gating + RMSNorm (channel-major) + o_proj, batched ----
    for ci in range(NCH):
        sl = slice(ci * C, (ci + 1) * C)
        O = o_all[:, ci * 256:(ci + 1) * 256]
        ot_ps = rtile()
        nc.tensor.transpose(ot_ps[:, 0:128], O[:, 0:128], eye[:])
        nc.tensor.transpose(ot_ps[:, 128:256], O[:, 128:256], eye[:])
        hT = rs.tile([128, 256], F32, tag="hT")
        nc.vector.tensor_mul(hT[:, 0:128], ot_ps[:, 0:128], zs0[:, sl])
        nc.vector.tensor_mul(hT[:, 128:256], ot_ps[:, 128:256], zs1[:, sl])
        hsq = rs.tile([128, 256], F32, tag="hsq")
        nc.vector.tensor_mul(hsq[:], hT[:], hT[:])
        ss_ps = ttile([1, 256])
        mm(ss_ps[:], onesc[:], hsq[:])
        rsg = rs.tile([1, 256], F32, tag="rsg")
        act(rsg[:], ss_ps[:], AF.Sqrt, bias=epsc[0:1, :], scale=1.0 / DV)
        nc.vector.reciprocal(rsg[:], rsg[:])
        rep_ps = rtile()
        mm(rep_ps[:], onesr[:], rsg[:])
        hn_ = rs.tile([128, 256], F32, tag="hn_")
        nc.vector.tensor_mul(hn_[:], hT[:], rep_ps[:])
        hnb = rs.tile([128, 256], BF16, tag="hnb")
        nc.vector.tensor_scalar_mul(hnb[:], hn_[:], normw[:])

        for n in range(4):
            op_ps = psb.tile([128, 512], F32, tag="A")
            ns = slice(n * 512, (n + 1) * 512)
            mm(op_ps[:], hnb[:, 0:128], wo[:, 0, ns], start=True, stop=False)
            mm(op_ps[:], hnb[:, 128:256], wo[:, 1, ns], start=False, stop=True)
            osb = ob.tile([128, 512], F32, tag="osb")
            nc.vector.tensor_copy(osb[:], op_ps[:])
            nc.sync.dma_start(out[sl, ns], osb[:])


_NC_CACHE = None


def _get_nc():
    global _NC_CACHE
    if _NC_CACHE is None:
        _NC_CACHE = _build_nc()
    return _NC_CACHE


def make_in_maps(x, w_qkvz, w_ba, conv_w, conv_b, a_log, dt_bias, norm_w, w_o):
    x = np.asarray(x, np.float32)
    xT = np.ascontiguousarray(x[0].T).astype(ml_dtypes.bfloat16)
    w_qkvz = np.asarray(w_qkvz, np.float32)
    w_ba = np.asarray(w_ba, np.float32)
    conv_w = np.asarray(conv_w, np.float32)
    conv_b = np.asarray(conv_b, np.float32)
    a_log = np.asarray(a_log, np.float32)
    dt_bias = np.asarray(dt_bias, np.float32)
    norm_w = np.asarray(norm_w, np.float32)
    w_o = np.asarray(w_o, np.float32)

    tri = np.triu(np.ones((128, 128), np.float32))          # j<=i in [j,i]
    consts = {
        "eye": np.eye(128, dtype=np.float32),
        "cums": tri.copy(),
        "mincl": np.tile(tri, (1, 2)).astype(np.float32),
        "msneg": np.tile(-np.triu(np.ones((128, 128), np.float32), 1),
                         (1, 2)).astype(np.float32),
        "onesc": np.ones((128, 1), np.float32),
        "onesr": np.ones((1, 128), np.float32),
        "qscr": np.full((1, 128), DK ** -0.5, np.float32),
        "epsc": np.full((128, 1), EPS, np.float32),
        "normw": norm_w.reshape(128, 1).astype(np.float32),
    }
    in_maps = []
    for c in range(NCORE):
        qs = slice(c * 128, (c + 1) * 128)
        ks = slice(KD + c * 128, KD + (c + 1) * 128)
        vs = slice(2 * KD + c * 256, 2 * KD + (c + 1) * 256)
        zs = slice(2 * KD + VD + c * 256, 2 * KD + VD + (c + 1) * 256)
        bachunk = np.zeros((H, 128), np.float32)
        bachunk[:, 0] = w_ba[:, 2 * c]
        bachunk[:, 1] = w_ba[:, 2 * c + 1]
        bachunk[:, 32] = w_ba[:, 2 * c + 1]
        bachunk[:, 64] = w_ba[:, HV + 2 * c]
        bachunk[:, 65] = w_ba[:, HV + 2 * c + 1]
        wq = np.concatenate([
            w_qkvz[:, qs], w_qkvz[:, ks], w_qkvz[:, vs], w_qkvz[:, zs],
            bachunk], axis=1).astype(ml_dtypes.bfloat16)
        chrows = np.r_[c * 128:(c + 1) * 128,
                       KD + c * 128:KD + (c + 1) * 128,
                       2 * KD + c * 256:2 * KD + (c + 1) * 256]
        m = {
            "xT": xT,
            "wq": wq,
            "convw": np.ascontiguousarray(conv_w[chrows]),
            "convb": np.ascontiguousarray(conv_b[chrows]).reshape(512, 1),
            "negea": (-np.exp(a_log[2 * c:2 * c + 2])).reshape(2, 1),
            "dtb": dt_bias[2 * c:2 * c + 2].reshape(2, 1).astype(np.float32),
            "wo": np.ascontiguousarray(
                w_o[c * 256:(c + 1) * 256]).astype(ml_dtypes.bfloat16),
        }
        m.update(consts)
        in_maps.append(m)
    return in_maps


def kernel(x, w_qkvz, w_ba, conv_w, conv_b, a_log, dt_bias, norm_w, w_o,
           **run_kwargs):
    in_maps = make_in_maps(x, w_qkvz, w_ba, conv_w, conv_b, a_log, dt_bias,
                           norm_w, w_o)
    nc = _get_nc()
    res = run_bass_kernel_spmd(nc, in_maps, core_ids=list(range(NCORE)),
                               **run_kwargs)
    acc = np.zeros((T, H), np.float32)
    for c in range(NCORE):
        acc += np.asarray(res.results[c]["out_part"], np.float32)
    if run_kwargs:
        kernel.last_results = res
    return acc.reshape(B, T, H)
